# revision 33
# baseline (speedup 1.0000x reference)
"""Trainium2 Bass kernel for nn_PeriodicSetTransformerEncoder.

Design (1.75x over the prior baseline: ~305us vs 538us on 8 cores):
- All projections computed DIRECTLY from x with host-fused weights.
  Since E=128 < D=512, q = x @ ((Wq@wq_W)*sc @ emb_W).T + bq2 etc.
  collapses the matmul contraction dim from 512 to 128: each of
  xe/q/k/v is 4 matmuls (K=128, F=512) per 4-example unit.
- All matmul operands bf16 (FWL weight loads; fp32 PSUM accumulate).
  Measured rel err 6.5e-3 vs the 2e-2 budget.
- Bias folds: k-bias cancels in softmax (adds a per-row constant);
  q-bias host-fused into bq2 (applied on the q PSUM drain); v-bias
  applied as the per-partition ACT bias of the softplus-exp (attention
  rows sum to 1 after renormalization); emb_b on the xe drain; LN
  affine folded into out_W (Wg/c1); cb added on HOST after gather.
- Split PSUM pools so the tile scheduler can overlap units (a single
  shared pool serializes stage allocation in emission order):
  ppj: projections (2-bank tiles, bufs=2) = 4 banks
  pat: scores (1-bank per-w tiles, bufs=2) = 2 banks
  ptl: transpose/att/LN/out (1-bank, bufs=2) = 2 banks
- Engine assignment tuned against the NTFF profile; the wall is the
  per-unit dependency chain threading DVE/ACT (both ~82% busy), so
  chain ops must stay on the fast engines (gpsimd TT is 0.42-eff and
  costs ~16x its extra latency in wall time).

Math (per example, N=128 tokens, E=128, D=512, H=4 heads, hd=128):
  xe = x@Memb.T + emb_b;  q = x@Mq.T + bq2;  k = x@Mk.T;  v = x@Mv.T
  s_h = q_h k_h^T;  e = exp(s);  U = sum_h e_h/rowsum(e_h)
  A = (U * w_j) / rowsum(U * w_j);  att = A @ v
  h = xe + softplus(att + bv2);  out = ((Wg.T h) - c1*mu) * rstd  (+ cb)

Sharding: pure data parallel, batch 512 -> 64 examples per core,
16 units of W=4 examples; features on partitions, tokens on free dim.
"""

import numpy as np

import concourse.bass as bass
import concourse.tile as tile
from concourse import bacc, mybir
from concourse.bass_utils import run_bass_kernel_spmd

F32 = mybir.dt.float32
F32R = mybir.dt.float32r
BF16 = mybir.dt.bfloat16
AX = mybir.AxisListType
OP = mybir.AluOpType
AF = mybir.ActivationFunctionType

B = 512
N = 128
E = 128
D = 512
H = 4
NCORES = 8
BC = B // NCORES          # examples per core
W = 4                     # examples per work unit (free-dim batching)
NU = BC // W              # work units per core
FD = W * N                # moving free dim for the big matmuls (512)


def build_nc(nu=NU):
    nc = bacc.Bacc("TRN2", target_bir_lowering=False, debug=False)

    xg = nc.dram_tensor("xg", [nu, 128, W, N], BF16, kind="ExternalInput").ap()
    wg = nc.dram_tensor("wg", [nu, W, N], BF16, kind="ExternalInput").ap()
    MembT = nc.dram_tensor("MembT", [128, 4, 128], BF16, kind="ExternalInput").ap()
    MqT = nc.dram_tensor("MqT", [128, 4, 128], BF16, kind="ExternalInput").ap()
    MkT = nc.dram_tensor("MkT", [128, 4, 128], BF16, kind="ExternalInput").ap()
    MvT = nc.dram_tensor("MvT", [128, 512], BF16, kind="ExternalInput").ap()
    WgT = nc.dram_tensor("WgT", [128, 4, 128], BF16, kind="ExternalInput").ap()
    c1n = nc.dram_tensor("c1n", [1, 128], BF16, kind="ExternalInput").ap()
    embb = nc.dram_tensor("embb", [128, 4], F32, kind="ExternalInput").ap()
    bq2 = nc.dram_tensor("bq2", [128, 4], F32, kind="ExternalInput").ap()
    bv2 = nc.dram_tensor("bv2", [128, 4], F32, kind="ExternalInput").ap()
    idm = nc.dram_tensor("idm", [128, 128], BF16, kind="ExternalInput").ap()
    onesm = nc.dram_tensor("onesm", [128, 128], BF16, kind="ExternalInput").ap()
    yT = nc.dram_tensor("yT", [nu, 128, W, N], F32, kind="ExternalOutput").ap()

    with tile.TileContext(nc) as tc:
        kernel_body(tc, nu, xg, wg, MembT, MqT, MkT, MvT, WgT, c1n,
                    embb, bq2, bv2, idm, onesm, yT)
    # All transcendentals (exp/ln/square) live in the
    # "natural_log_exp_and_others" activation table set.  Restrict the
    # table map during compile so the act-table-load pass emits a single
    # load instead of thrashing between per-function sets.
    from concourse import hw_specs
    orig = hw_specs.get_activation_tables

    def patched(arch):
        t = orig(arch)
        strip = {AF.Exp, AF.Ln, AF.Square}
        for name, fs in t.items():
            if name != "natural_log_exp_and_others":
                t[name] = fs - strip
        return t

    hw_specs.get_activation_tables = patched
    bacc_mod = __import__("concourse.bacc", fromlist=["get_activation_tables"])
    had = getattr(bacc_mod, "get_activation_tables", None)
    if had is not None:
        bacc_mod.get_activation_tables = patched
    try:
        nc.compile()
    finally:
        hw_specs.get_activation_tables = orig
        if had is not None:
            bacc_mod.get_activation_tables = had
    return nc


def kernel_body(tc, nu, xg, wg, MembT, MqT, MkT, MvT, WgT, c1n,
                embb, bq2, bv2, idm, onesm, yT):
    nc = tc.nc
    from contextlib import ExitStack
    ctx = ExitStack()
    with ctx:
        const = ctx.enter_context(tc.tile_pool(name="const", bufs=1))
        ppj = ctx.enter_context(tc.tile_pool(name="ppj", bufs=2, space="PSUM"))
        pat = ctx.enter_context(tc.tile_pool(name="pat", bufs=1, space="PSUM"))
        ptl = ctx.enter_context(tc.tile_pool(name="ptl", bufs=2, space="PSUM"))
        xpool = ctx.enter_context(tc.tile_pool(name="xpool", bufs=6))
        xepool = ctx.enter_context(tc.tile_pool(name="xepool", bufs=3))
        qkpool = ctx.enter_context(tc.tile_pool(name="qkpool", bufs=4))
        vpool = ctx.enter_context(tc.tile_pool(name="vpool", bufs=4))
        epool = ctx.enter_context(tc.tile_pool(name="epool", bufs=4))
        spool = ctx.enter_context(tc.tile_pool(name="spool", bufs=4))
        hpool = ctx.enter_context(tc.tile_pool(name="hpool", bufs=3))
        tiny = ctx.enter_context(tc.tile_pool(name="tiny", bufs=6))
        opool = ctx.enter_context(tc.tile_pool(name="opool", bufs=4))

        # ---- constants ----
        ident = const.tile([128, 128], BF16)
        nc.sync.dma_start(ident, idm)
        ones = const.tile([128, 128], BF16)
        nc.sync.dma_start(ones, onesm)
        MembT_s = const.tile([128, 4, 128], BF16)
        nc.sync.dma_start(MembT_s, MembT)
        MqT_s = const.tile([128, 4, 128], BF16)
        nc.sync.dma_start(MqT_s, MqT)
        MkT_s = const.tile([128, 4, 128], BF16)
        nc.sync.dma_start(MkT_s, MkT)
        MvT_s = const.tile([128, 512], BF16)
        nc.sync.dma_start(MvT_s, MvT)
        WgT_s = const.tile([128, 4, 128], BF16)
        nc.sync.dma_start(WgT_s, WgT)
        c1n_s = const.tile([1, 128], BF16)
        nc.sync.dma_start(c1n_s, c1n)
        embb_s = const.tile([128, 4], F32)
        nc.sync.dma_start(embb_s, embb)
        bq2_s = const.tile([128, 4], F32)
        nc.sync.dma_start(bq2_s, bq2)
        bv2_s = const.tile([128, 4], F32)
        nc.sync.dma_start(bv2_s, bv2)
        eps = const.tile([128, 1], F32)
        nc.vector.memset(eps, 1e-5)
        one_b = const.tile([128, 1], F32)
        nc.vector.memset(one_b, 1.0)

        for u in range(nu):
            unit_body(nc, u, xg, wg, yT,
                      MembT_s, MqT_s, MkT_s, MvT_s, WgT_s, c1n_s,
                      embb_s, bq2_s, bv2_s, eps, one_b, ident, ones,
                      ppj, pat, ptl, xpool, xepool, qkpool, vpool, epool,
                      spool, hpool, tiny, opool)


def unit_body(nc, u, xg, wg, yT,
              MembT_s, MqT_s, MkT_s, MvT_s, WgT_s, c1n_s,
              embb_s, bq2_s, bv2_s, eps, one_b, ident, ones,
              ppj, pat, ptl, xpool, xepool, qkpool, vpool, epool,
              spool, hpool, tiny, opool):
    ts = bass.ts

    # ---- load x (transposed, bf16) and per-token weights ----
    xT = xpool.tile([128, W, N], BF16, tag="xT")
    nc.sync.dma_start(xT, xg[u])
    wrow = xpool.tile([128, W, N], BF16, tag="wrow")
    nc.gpsimd.dma_start(wrow, wg[u : u + 1].to_broadcast((128, W, N)))

    # ---- projections from x: all K=128 ----
    xeT = xepool.tile([128, 4, FD], BF16, tag="xeT")
    qT = qkpool.tile([128, 4, W, N], BF16, tag="qT")
    kT = qkpool.tile([128, 4, W, N], BF16, tag="kT")
    v = vpool.tile([128, W, 512], BF16, tag="v")
    for hh in range(2):
        pxe = ppj.tile([128, 2, FD], F32, tag="bank", name=f"pxe_{u}_{hh}")
        for c2 in range(2):
            nc.tensor.matmul(pxe[:, c2], MembT_s[:, hh * 2 + c2], xT,
                             start=True, stop=True)
        nc.vector.tensor_add(
            xeT[:, hh * 2 : hh * 2 + 2], pxe,
            embb_s[:, hh * 2 : hh * 2 + 2, None].to_broadcast((128, 2, FD)))
        pq = ppj.tile([128, 2, FD], F32, tag="bank", name=f"pq_{u}_{hh}")
        for c2 in range(2):
            nc.tensor.matmul(pq[:, c2], MqT_s[:, hh * 2 + c2], xT,
                             start=True, stop=True)
        nc.vector.tensor_add(
            qT[:, hh * 2 : hh * 2 + 2], pq,
            bq2_s[:, hh * 2 : hh * 2 + 2, None].to_broadcast((128, 2, FD)))
        pk = ppj.tile([128, 2, FD], F32, tag="bank", name=f"pk_{u}_{hh}")
        for c2 in range(2):
            nc.tensor.matmul(pk[:, c2], MkT_s[:, hh * 2 + c2], xT,
                             start=True, stop=True)
        nc.scalar.copy(kT[:, hh * 2 : hh * 2 + 2], pk)
        pv = ppj.tile([128, 2, 512], F32, tag="bank", name=f"pv_{u}_{hh}")
        for w_i in range(2):
            nc.tensor.matmul(pv[:, w_i], xT[:, hh * 2 + w_i], MvT_s,
                             start=True, stop=True)
        nc.scalar.copy(v[:, hh * 2 : hh * 2 + 2], pv)

    # ---- attention scores + exp ----
    e_all = epool.tile([128, W, H, N], BF16, tag="e_all")
    for hh in range(2):
        pss = pat.tile([128, 2, H, N], F32, tag="bank", name=f"pss_{u}_{hh}")
        for w_i in range(2):
            for h in range(H):
                nc.tensor.matmul(pss[:, w_i, h], qT[:, h, hh * 2 + w_i],
                                 kT[:, h, hh * 2 + w_i], start=True, stop=True)
        nc.scalar.activation(e_all[:, hh * 2 : hh * 2 + 2], pss, AF.Exp)

    # ---- softmax normalize (per head), head-sum, token weights ----
    s_all = tiny.tile([128, W, H], F32, tag="s_all")
    nc.vector.reduce_sum(s_all, e_all, axis=AX.X)
    r_all = tiny.tile([128, W, H], BF16, tag="r_all")
    with nc.allow_low_precision(reason="softmax normalizer in bf16 is fine"):
        nc.vector.reciprocal(r_all, s_all)
    nc.vector.tensor_mul(e_all, e_all,
                         r_all[:, :, :, None].to_broadcast((128, W, H, N)))
    nc.vector.tensor_add(e_all[:, :, 0:2], e_all[:, :, 0:2], e_all[:, :, 2:4])
    Sw = spool.tile([128, W, N], BF16, tag="Sw")
    nc.vector.tensor_add(Sw, e_all[:, :, 0], e_all[:, :, 1])
    Sww = spool.tile([128, W, N], BF16, tag="Sww")
    nc.vector.tensor_mul(Sww, Sw, wrow)
    dd = tiny.tile([128, W], F32, tag="dd")
    nc.vector.reduce_sum(dd, Sww, axis=AX.X)
    rd = tiny.tile([128, W], BF16, tag="rd")
    with nc.allow_low_precision(reason="attention renormalizer in bf16 is fine"):
        nc.vector.reciprocal(rd, dd)
    Ab = spool.tile([128, W, N], BF16, tag="Ab")
    nc.vector.tensor_mul(Ab, Sww, rd[:, :, None].to_broadcast((128, W, N)))

    # ---- transpose A, att = A @ v  (att^T in PSUM) ----
    pT = ptl.tile([128, W, N], BF16, tag="bank", name=f"pT_{u}")
    for w_i in range(W):
        nc.tensor.transpose(pT[:, w_i], Ab[:, w_i], ident)
    awT = spool.tile([128, W, N], BF16, tag="awT")
    nc.scalar.copy(awT, pT)

    spx = hpool.tile([128, 4, FD], BF16, tag="spx")
    for c in range(4):
        pa = ptl.tile([128, W, N], F32, tag="bank", name=f"pa_{u}_{c}")
        for w_i in range(W):
            nc.tensor.matmul(pa[:, w_i],
                             v[:, w_i, ts(c, 128)], awT[:, w_i],
                             start=True, stop=True)
        # softplus part 1: exp(att + bv2)  (v bias folded in here)
        nc.scalar.activation(spx[:, c], pa, AF.Exp,
                             bias=bv2_s[:, c : c + 1])
    # softplus part 2: ln(exp + 1); then h = xe + softplus
    ea = hpool.tile([128, 4, FD], BF16, tag="ea")
    nc.scalar.activation(ea, spx, AF.Ln, bias=one_b)
    hT = hpool.tile([128, 4, FD], BF16, tag="hT")
    nc.vector.tensor_add(hT, ea, xeT)
    sq = hpool.tile([128, 4, FD], BF16, tag="sq")
    nc.scalar.square(sq, hT)

    # ---- LayerNorm stats over d via ones-matmul ----
    ps_s = ptl.tile([128, FD], F32, tag="bank", name=f"ps_s_{u}")
    for c in range(4):
        nc.tensor.matmul(ps_s, ones, hT[:, c], start=(c == 0), stop=(c == 3))
    ps_q = ptl.tile([128, FD], F32, tag="bank", name=f"ps_q_{u}")
    for c in range(4):
        nc.tensor.matmul(ps_q, ones, sq[:, c], start=(c == 0), stop=(c == 3))
    # mu2 = (s/512)^2 ; var = q/512 - mu2 ; rstd = exp(-0.5*ln(var+eps))
    mu2 = spool.tile([128, FD], F32, tag="mu2")
    nc.scalar.activation(mu2, ps_s, AF.Square, scale=1.0 / D)
    var = spool.tile([128, FD], F32, tag="var")
    nc.vector.scalar_tensor_tensor(out=var, in0=ps_q, scalar=1.0 / D,
                                   in1=mu2, op0=OP.mult, op1=OP.subtract)
    lv = spool.tile([128, FD], F32, tag="lv")
    nc.scalar.activation(lv, var, AF.Ln, bias=eps)
    rstd = spool.tile([128, FD], F32, tag="rstd")
    nc.scalar.activation(rstd, lv, AF.Exp, scale=-0.5)
    m2 = tiny.tile([1, FD], BF16, tag="m2")
    nc.vector.tensor_scalar_mul(m2, ps_s[0:1], 1.0 / D)

    # ---- out^T = Wg-chunks @ h^T - c1 x mu ; then *rstd (cb on host) ----
    po = ptl.tile([128, FD], F32, tag="bank", name=f"po_{u}")
    for c in range(4):
        nc.tensor.matmul(po, WgT_s[:, c], hT[:, c], start=(c == 0), stop=False)
    nc.tensor.matmul(po, c1n_s, m2, start=False, stop=True)
    outT = opool.tile([128, W, N], F32, tag="outT")
    nc.vector.tensor_mul(outT, po, rstd)
    nc.sync.dma_start(yT[u], outT)


# ------------------------- host side -------------------------

def host_prep(x, weights, emb_W, emb_b, wq_W, wq_b, wk_W, wk_b, wv_W, wv_b,
              in_proj_W, in_proj_b, ln_g, ln_b, out_W, out_b):
    """Fuse/reshape parameters and build per-core input maps."""
    import ml_dtypes
    f = np.float32
    bf = ml_dtypes.bfloat16
    sc = 1.0 / np.sqrt(np.float32(E))

    Wq = in_proj_W[:D]
    Wk = in_proj_W[D : 2 * D]
    bqi = in_proj_b[:D]
    Wqc = (Wq @ wq_W) * sc
    bq_eff = (Wq @ wq_b + bqi) * sc
    Wkc = Wk @ wk_W

    Mq = Wqc @ emb_W                       # [D, E]
    bq2v = Wqc @ emb_b + bq_eff            # [D]
    Mk = Wkc @ emb_W
    Mv = wv_W @ emb_W
    bv2v = wv_W @ emb_b + wv_b             # [D]

    Wg = out_W.T * ln_g[:, None]           # [D, E]
    c1 = Wg.sum(axis=0)                    # [E]
    cbv = out_b + out_W @ ln_b             # [E]

    def chunkT(M):
        # M [D, E] applied as x @ M.T -> stationary chunks [128(E), 4, 128]
        return np.ascontiguousarray(M.T.reshape(128, 4, 128)).astype(bf)

    def pcol(vec):
        return np.ascontiguousarray(vec.reshape(4, 128).T).astype(f)

    params = {
        "MembT": np.ascontiguousarray(emb_W.T.reshape(128, 4, 128)).astype(bf),
        "MqT": chunkT(Mq),
        "MkT": chunkT(Mk),
        "MvT": np.ascontiguousarray(Mv.T).astype(bf),   # [E, D] moving
        "WgT": np.ascontiguousarray(
            Wg.reshape(4, 128, 128).transpose(1, 0, 2)).astype(bf),
        "c1n": np.ascontiguousarray((-c1).reshape(1, 128)).astype(bf),
        "embb": pcol(emb_b),
        "bq2": pcol(bq2v),
        "bv2": pcol(bv2v),
        "onesm": np.ones((128, 128), dtype=bf),
        "idm": np.eye(128).astype(bf),
    }

    in_maps = []
    for c in range(NCORES):
        xs = x[c * BC : (c + 1) * BC].astype(f)
        ws = weights[c * BC : (c + 1) * BC, :, 0].astype(f)
        xgc = np.ascontiguousarray(
            xs.reshape(NU, W, N, E).transpose(0, 3, 1, 2)).astype(bf)
        wgc = np.ascontiguousarray(ws.reshape(NU, W, N)).astype(bf)
        m = dict(params)
        m["xg"] = xgc
        m["wg"] = wgc
        in_maps.append(m)
    return in_maps, cbv


_NC_CACHE = {}


def kernel(**inputs):
    key = "full"
    if key not in _NC_CACHE:
        _NC_CACHE[key] = build_nc(NU)
    nc = _NC_CACHE[key]
    in_maps, cbv = host_prep(**inputs)
    res = run_bass_kernel_spmd(nc, in_maps, core_ids=list(range(NCORES)))
    outs = []
    for c in range(NCORES):
        yt = res.results[c]["yT"]                  # [NU, 128(E), W, N]
        y = yt.transpose(0, 2, 3, 1).reshape(BC, N, E)
        outs.append(y)
    full = np.concatenate(outs, axis=0) + cbv[None, None, :]
    return np.ascontiguousarray(full).astype(np.float32)


# revision 35
# speedup vs baseline: 1.0005x; 1.0005x over previous
"""Trainium2 Bass kernel for nn_PeriodicSetTransformerEncoder.

Design (1.75x over the prior baseline: ~305us vs 538us on 8 cores):
- All projections computed DIRECTLY from x with host-fused weights.
  Since E=128 < D=512, q = x @ ((Wq@wq_W)*sc @ emb_W).T + bq2 etc.
  collapses the matmul contraction dim from 512 to 128: each of
  xe/q/k/v is 4 matmuls (K=128, F=512) per 4-example unit.
- All matmul operands bf16 (FWL weight loads; fp32 PSUM accumulate).
  Measured rel err 6.5e-3 vs the 2e-2 budget.
- Bias folds: k-bias cancels in softmax (adds a per-row constant);
  q-bias host-fused into bq2 (applied on the q PSUM drain); v-bias
  applied as the per-partition ACT bias of the softplus-exp (attention
  rows sum to 1 after renormalization); emb_b on the xe drain; LN
  affine folded into out_W (Wg/c1); cb added on HOST after gather.
- Split PSUM pools so the tile scheduler can overlap units (a single
  shared pool serializes stage allocation in emission order):
  ppj: projections (2-bank tiles, bufs=2) = 4 banks
  pat: scores (1-bank per-w tiles, bufs=2) = 2 banks
  ptl: transpose/att/LN/out (1-bank, bufs=2) = 2 banks
- Engine assignment tuned against the NTFF profile; the wall is the
  per-unit dependency chain threading DVE/ACT (both ~82% busy), so
  chain ops must stay on the fast engines (gpsimd TT is 0.42-eff and
  costs ~16x its extra latency in wall time).

Math (per example, N=128 tokens, E=128, D=512, H=4 heads, hd=128):
  xe = x@Memb.T + emb_b;  q = x@Mq.T + bq2;  k = x@Mk.T;  v = x@Mv.T
  s_h = q_h k_h^T;  e = exp(s);  U = sum_h e_h/rowsum(e_h)
  A = (U * w_j) / rowsum(U * w_j);  att = A @ v
  h = xe + softplus(att + bv2);  out = ((Wg.T h) - c1*mu) * rstd  (+ cb)

Sharding: pure data parallel, batch 512 -> 64 examples per core,
16 units of W=4 examples; features on partitions, tokens on free dim.
"""

import numpy as np

import concourse.bass as bass
import concourse.tile as tile
from concourse import bacc, mybir
from concourse.bass_utils import run_bass_kernel_spmd

F32 = mybir.dt.float32
F32R = mybir.dt.float32r
BF16 = mybir.dt.bfloat16
AX = mybir.AxisListType
OP = mybir.AluOpType
AF = mybir.ActivationFunctionType

B = 512
N = 128
E = 128
D = 512
H = 4
NCORES = 8
BC = B // NCORES          # examples per core
W = 4                     # examples per work unit (free-dim batching)
NU = BC // W              # work units per core
FD = W * N                # moving free dim for the big matmuls (512)


def build_nc(nu=NU):
    nc = bacc.Bacc("TRN2", target_bir_lowering=False, debug=False)

    xg = nc.dram_tensor("xg", [nu, 128, W, N], BF16, kind="ExternalInput").ap()
    wg = nc.dram_tensor("wg", [nu, W, N], BF16, kind="ExternalInput").ap()
    MembT = nc.dram_tensor("MembT", [128, 4, 128], BF16, kind="ExternalInput").ap()
    MqT = nc.dram_tensor("MqT", [128, 4, 128], BF16, kind="ExternalInput").ap()
    MkT = nc.dram_tensor("MkT", [128, 4, 128], BF16, kind="ExternalInput").ap()
    MvT = nc.dram_tensor("MvT", [128, 512], BF16, kind="ExternalInput").ap()
    WgT = nc.dram_tensor("WgT", [128, 4, 128], BF16, kind="ExternalInput").ap()
    c1n = nc.dram_tensor("c1n", [1, 128], BF16, kind="ExternalInput").ap()
    embb = nc.dram_tensor("embb", [128, 4], F32, kind="ExternalInput").ap()
    bq2 = nc.dram_tensor("bq2", [128, 4], F32, kind="ExternalInput").ap()
    bv2 = nc.dram_tensor("bv2", [128, 4], F32, kind="ExternalInput").ap()
    idm = nc.dram_tensor("idm", [128, 128], BF16, kind="ExternalInput").ap()
    onesm = nc.dram_tensor("onesm", [128, 128], BF16, kind="ExternalInput").ap()
    yT = nc.dram_tensor("yT", [nu, 128, W, N], F32, kind="ExternalOutput").ap()

    with tile.TileContext(nc) as tc:
        kernel_body(tc, nu, xg, wg, MembT, MqT, MkT, MvT, WgT, c1n,
                    embb, bq2, bv2, idm, onesm, yT)
    # All transcendentals (exp/ln/square) live in the
    # "natural_log_exp_and_others" activation table set.  Restrict the
    # table map during compile so the act-table-load pass emits a single
    # load instead of thrashing between per-function sets.
    from concourse import hw_specs
    orig = hw_specs.get_activation_tables

    def patched(arch):
        t = orig(arch)
        strip = {AF.Exp, AF.Ln, AF.Square}
        for name, fs in t.items():
            if name != "natural_log_exp_and_others":
                t[name] = fs - strip
        return t

    hw_specs.get_activation_tables = patched
    bacc_mod = __import__("concourse.bacc", fromlist=["get_activation_tables"])
    had = getattr(bacc_mod, "get_activation_tables", None)
    if had is not None:
        bacc_mod.get_activation_tables = patched
    try:
        nc.compile()
    finally:
        hw_specs.get_activation_tables = orig
        if had is not None:
            bacc_mod.get_activation_tables = had
    return nc


def kernel_body(tc, nu, xg, wg, MembT, MqT, MkT, MvT, WgT, c1n,
                embb, bq2, bv2, idm, onesm, yT):
    nc = tc.nc
    from contextlib import ExitStack
    ctx = ExitStack()
    with ctx:
        const = ctx.enter_context(tc.tile_pool(name="const", bufs=1))
        ppj = ctx.enter_context(tc.tile_pool(name="ppj", bufs=2, space="PSUM"))
        pat = ctx.enter_context(tc.tile_pool(name="pat", bufs=1, space="PSUM"))
        ptl = ctx.enter_context(tc.tile_pool(name="ptl", bufs=2, space="PSUM"))
        pdum = ctx.enter_context(tc.tile_pool(name="pdum", bufs=1, space="PSUM"))
        xpool = ctx.enter_context(tc.tile_pool(name="xpool", bufs=6))
        xepool = ctx.enter_context(tc.tile_pool(name="xepool", bufs=3))
        qkpool = ctx.enter_context(tc.tile_pool(name="qkpool", bufs=4))
        vpool = ctx.enter_context(tc.tile_pool(name="vpool", bufs=4))
        epool = ctx.enter_context(tc.tile_pool(name="epool", bufs=4))
        spool = ctx.enter_context(tc.tile_pool(name="spool", bufs=4))
        hpool = ctx.enter_context(tc.tile_pool(name="hpool", bufs=3))
        tiny = ctx.enter_context(tc.tile_pool(name="tiny", bufs=6))
        opool = ctx.enter_context(tc.tile_pool(name="opool", bufs=4))

        # ---- constants ----
        ident = const.tile([128, 128], BF16)
        nc.sync.dma_start(ident, idm)
        ones = const.tile([128, 128], BF16)
        nc.sync.dma_start(ones, onesm)
        MembT_s = const.tile([128, 4, 128], BF16)
        nc.sync.dma_start(MembT_s, MembT)
        MqT_s = const.tile([128, 4, 128], BF16)
        nc.sync.dma_start(MqT_s, MqT)
        MkT_s = const.tile([128, 4, 128], BF16)
        nc.sync.dma_start(MkT_s, MkT)
        MvT_s = const.tile([128, 512], BF16)
        nc.sync.dma_start(MvT_s, MvT)
        WgT_s = const.tile([128, 4, 128], BF16)
        nc.sync.dma_start(WgT_s, WgT)
        c1n_s = const.tile([1, 128], BF16)
        nc.sync.dma_start(c1n_s, c1n)
        embb_s = const.tile([128, 4], F32)
        nc.sync.dma_start(embb_s, embb)
        bq2_s = const.tile([128, 4], F32)
        nc.sync.dma_start(bq2_s, bq2)
        bv2_s = const.tile([128, 4], F32)
        nc.sync.dma_start(bv2_s, bv2)
        eps = const.tile([128, 1], F32)
        nc.vector.memset(eps, 1e-5)
        one_b = const.tile([128, 1], F32)
        nc.vector.memset(one_b, 1.0)

        for u in range(nu):
            unit_body(nc, u, xg, wg, yT,
                      MembT_s, MqT_s, MkT_s, MvT_s, WgT_s, c1n_s,
                      embb_s, bq2_s, bv2_s, eps, one_b, ident, ones,
                      ppj, pat, ptl, xpool, xepool, qkpool, vpool, epool,
                      spool, hpool, tiny, opool)
        # HAM warm-keepers: lowest-priority (emitted last) dummy matmuls
        # that the scheduler slots into PE idle gaps, holding the PE clock
        # at 2.4 GHz so the real chain matmuls never run throttled.
        pdt = pdum.tile([128, FD], F32, tag="dummy")
        for i in range(200):
            nc.tensor.matmul(pdt, ones, MvT_s, start=True, stop=True)


def unit_body(nc, u, xg, wg, yT,
              MembT_s, MqT_s, MkT_s, MvT_s, WgT_s, c1n_s,
              embb_s, bq2_s, bv2_s, eps, one_b, ident, ones,
              ppj, pat, ptl, xpool, xepool, qkpool, vpool, epool,
              spool, hpool, tiny, opool):
    ts = bass.ts

    # ---- load x (transposed, bf16) and per-token weights ----
    xT = xpool.tile([128, W, N], BF16, tag="xT")
    nc.sync.dma_start(xT, xg[u])
    wrow = xpool.tile([128, W, N], BF16, tag="wrow")
    nc.gpsimd.dma_start(wrow, wg[u : u + 1].to_broadcast((128, W, N)))

    # ---- projections from x: all K=128 ----
    xeT = xepool.tile([128, 4, FD], BF16, tag="xeT")
    qT = qkpool.tile([128, 4, W, N], BF16, tag="qT")
    kT = qkpool.tile([128, 4, W, N], BF16, tag="kT")
    v = vpool.tile([128, W, 512], BF16, tag="v")
    for hh in range(2):
        pxe = ppj.tile([128, 2, FD], F32, tag="bank", name=f"pxe_{u}_{hh}")
        for c2 in range(2):
            nc.tensor.matmul(pxe[:, c2], MembT_s[:, hh * 2 + c2], xT,
                             start=True, stop=True)
        nc.vector.tensor_add(
            xeT[:, hh * 2 : hh * 2 + 2], pxe,
            embb_s[:, hh * 2 : hh * 2 + 2, None].to_broadcast((128, 2, FD)))
        pq = ppj.tile([128, 2, FD], F32, tag="bank", name=f"pq_{u}_{hh}")
        for c2 in range(2):
            nc.tensor.matmul(pq[:, c2], MqT_s[:, hh * 2 + c2], xT,
                             start=True, stop=True)
        nc.vector.tensor_add(
            qT[:, hh * 2 : hh * 2 + 2], pq,
            bq2_s[:, hh * 2 : hh * 2 + 2, None].to_broadcast((128, 2, FD)))
        pk = ppj.tile([128, 2, FD], F32, tag="bank", name=f"pk_{u}_{hh}")
        for c2 in range(2):
            nc.tensor.matmul(pk[:, c2], MkT_s[:, hh * 2 + c2], xT,
                             start=True, stop=True)
        nc.scalar.copy(kT[:, hh * 2 : hh * 2 + 2], pk)
        pv = ppj.tile([128, 2, 512], F32, tag="bank", name=f"pv_{u}_{hh}")
        for w_i in range(2):
            nc.tensor.matmul(pv[:, w_i], xT[:, hh * 2 + w_i], MvT_s,
                             start=True, stop=True)
        nc.scalar.copy(v[:, hh * 2 : hh * 2 + 2], pv)

    # ---- attention scores + exp ----
    e_all = epool.tile([128, W, H, N], BF16, tag="e_all")
    for w_i in range(W):
        pss = pat.tile([128, H, N], F32, tag="bank", name=f"pss_{u}_{w_i}")
        for h in range(H):
            nc.tensor.matmul(pss[:, h], qT[:, h, w_i],
                             kT[:, h, w_i], start=True, stop=True)
        nc.scalar.activation(e_all[:, w_i], pss, AF.Exp)

    # ---- softmax normalize (per head), head-sum, token weights ----
    s_all = tiny.tile([128, W, H], F32, tag="s_all")
    nc.vector.reduce_sum(s_all, e_all, axis=AX.X)
    r_all = tiny.tile([128, W, H], BF16, tag="r_all")
    with nc.allow_low_precision(reason="softmax normalizer in bf16 is fine"):
        nc.vector.reciprocal(r_all, s_all)
    nc.vector.tensor_mul(e_all, e_all,
                         r_all[:, :, :, None].to_broadcast((128, W, H, N)))
    nc.vector.tensor_add(e_all[:, :, 0:2], e_all[:, :, 0:2], e_all[:, :, 2:4])
    Sw = spool.tile([128, W, N], BF16, tag="Sw")
    nc.vector.tensor_add(Sw, e_all[:, :, 0], e_all[:, :, 1])
    Sww = spool.tile([128, W, N], BF16, tag="Sww")
    nc.vector.tensor_mul(Sww, Sw, wrow)
    dd = tiny.tile([128, W], F32, tag="dd")
    nc.vector.reduce_sum(dd, Sww, axis=AX.X)
    rd = tiny.tile([128, W], BF16, tag="rd")
    with nc.allow_low_precision(reason="attention renormalizer in bf16 is fine"):
        nc.vector.reciprocal(rd, dd)
    Ab = spool.tile([128, W, N], BF16, tag="Ab")
    nc.vector.tensor_mul(Ab, Sww, rd[:, :, None].to_broadcast((128, W, N)))

    # ---- transpose A, att = A @ v  (att^T in PSUM) ----
    pT = ptl.tile([128, W, N], BF16, tag="bank", name=f"pT_{u}")
    for w_i in range(W):
        nc.tensor.transpose(pT[:, w_i], Ab[:, w_i], ident)
    awT = spool.tile([128, W, N], BF16, tag="awT")
    nc.scalar.copy(awT, pT)

    spx = hpool.tile([128, 4, FD], BF16, tag="spx")
    for c in range(4):
        pa = ptl.tile([128, W, N], F32, tag="bank", name=f"pa_{u}_{c}")
        for w_i in range(W):
            nc.tensor.matmul(pa[:, w_i],
                             v[:, w_i, ts(c, 128)], awT[:, w_i],
                             start=True, stop=True)
        # softplus part 1: exp(att + bv2)  (v bias folded in here)
        nc.scalar.activation(spx[:, c], pa, AF.Exp,
                             bias=bv2_s[:, c : c + 1])
    # softplus part 2: ln(exp + 1); then h = xe + softplus
    ea = hpool.tile([128, 4, FD], BF16, tag="ea")
    nc.scalar.activation(ea, spx, AF.Ln, bias=one_b)
    hT = hpool.tile([128, 4, FD], BF16, tag="hT")
    nc.vector.tensor_add(hT, ea, xeT)
    sq = hpool.tile([128, 4, FD], BF16, tag="sq")
    nc.scalar.square(sq, hT)

    # ---- LayerNorm stats over d via ones-matmul ----
    ps_s = ptl.tile([128, FD], F32, tag="bank", name=f"ps_s_{u}")
    for c in range(4):
        nc.tensor.matmul(ps_s, ones, hT[:, c], start=(c == 0), stop=(c == 3))
    ps_q = ptl.tile([128, FD], F32, tag="bank", name=f"ps_q_{u}")
    for c in range(4):
        nc.tensor.matmul(ps_q, ones, sq[:, c], start=(c == 0), stop=(c == 3))
    # mu2 = (s/512)^2 ; var = q/512 - mu2 ; rstd = exp(-0.5*ln(var+eps))
    mu2 = spool.tile([128, FD], F32, tag="mu2")
    nc.scalar.activation(mu2, ps_s, AF.Square, scale=1.0 / D)
    var = spool.tile([128, FD], F32, tag="var")
    nc.vector.scalar_tensor_tensor(out=var, in0=ps_q, scalar=1.0 / D,
                                   in1=mu2, op0=OP.mult, op1=OP.subtract)
    lv = spool.tile([128, FD], F32, tag="lv")
    nc.scalar.activation(lv, var, AF.Ln, bias=eps)
    rstd = spool.tile([128, FD], F32, tag="rstd")
    nc.scalar.activation(rstd, lv, AF.Exp, scale=-0.5)
    m2 = tiny.tile([1, FD], BF16, tag="m2")
    nc.vector.tensor_scalar_mul(m2, ps_s[0:1], 1.0 / D)

    # ---- out^T = Wg-chunks @ h^T - c1 x mu ; then *rstd (cb on host) ----
    po = ptl.tile([128, FD], F32, tag="bank", name=f"po_{u}")
    for c in range(4):
        nc.tensor.matmul(po, WgT_s[:, c], hT[:, c], start=(c == 0), stop=False)
    nc.tensor.matmul(po, c1n_s, m2, start=False, stop=True)
    outT = opool.tile([128, W, N], F32, tag="outT")
    nc.vector.tensor_mul(outT, po, rstd)
    nc.sync.dma_start(yT[u], outT)


# ------------------------- host side -------------------------

def host_prep(x, weights, emb_W, emb_b, wq_W, wq_b, wk_W, wk_b, wv_W, wv_b,
              in_proj_W, in_proj_b, ln_g, ln_b, out_W, out_b):
    """Fuse/reshape parameters and build per-core input maps."""
    import ml_dtypes
    f = np.float32
    bf = ml_dtypes.bfloat16
    sc = 1.0 / np.sqrt(np.float32(E))

    Wq = in_proj_W[:D]
    Wk = in_proj_W[D : 2 * D]
    bqi = in_proj_b[:D]
    Wqc = (Wq @ wq_W) * sc
    bq_eff = (Wq @ wq_b + bqi) * sc
    Wkc = Wk @ wk_W

    Mq = Wqc @ emb_W                       # [D, E]
    bq2v = Wqc @ emb_b + bq_eff            # [D]
    Mk = Wkc @ emb_W
    Mv = wv_W @ emb_W
    bv2v = wv_W @ emb_b + wv_b             # [D]

    Wg = out_W.T * ln_g[:, None]           # [D, E]
    c1 = Wg.sum(axis=0)                    # [E]
    cbv = out_b + out_W @ ln_b             # [E]

    def chunkT(M):
        # M [D, E] applied as x @ M.T -> stationary chunks [128(E), 4, 128]
        return np.ascontiguousarray(M.T.reshape(128, 4, 128)).astype(bf)

    def pcol(vec):
        return np.ascontiguousarray(vec.reshape(4, 128).T).astype(f)

    params = {
        "MembT": np.ascontiguousarray(emb_W.T.reshape(128, 4, 128)).astype(bf),
        "MqT": chunkT(Mq),
        "MkT": chunkT(Mk),
        "MvT": np.ascontiguousarray(Mv.T).astype(bf),   # [E, D] moving
        "WgT": np.ascontiguousarray(
            Wg.reshape(4, 128, 128).transpose(1, 0, 2)).astype(bf),
        "c1n": np.ascontiguousarray((-c1).reshape(1, 128)).astype(bf),
        "embb": pcol(emb_b),
        "bq2": pcol(bq2v),
        "bv2": pcol(bv2v),
        "onesm": np.ones((128, 128), dtype=bf),
        "idm": np.eye(128).astype(bf),
    }

    in_maps = []
    for c in range(NCORES):
        xs = x[c * BC : (c + 1) * BC].astype(f)
        ws = weights[c * BC : (c + 1) * BC, :, 0].astype(f)
        xgc = np.ascontiguousarray(
            xs.reshape(NU, W, N, E).transpose(0, 3, 1, 2)).astype(bf)
        wgc = np.ascontiguousarray(ws.reshape(NU, W, N)).astype(bf)
        m = dict(params)
        m["xg"] = xgc
        m["wg"] = wgc
        in_maps.append(m)
    return in_maps, cbv


_NC_CACHE = {}


def kernel(**inputs):
    key = "full"
    if key not in _NC_CACHE:
        _NC_CACHE[key] = build_nc(NU)
    nc = _NC_CACHE[key]
    in_maps, cbv = host_prep(**inputs)
    res = run_bass_kernel_spmd(nc, in_maps, core_ids=list(range(NCORES)))
    outs = []
    for c in range(NCORES):
        yt = res.results[c]["yT"]                  # [NU, 128(E), W, N]
        y = yt.transpose(0, 2, 3, 1).reshape(BC, N, E)
        outs.append(y)
    full = np.concatenate(outs, axis=0) + cbv[None, None, :]
    return np.ascontiguousarray(full).astype(np.float32)


# revision 36
# speedup vs baseline: 1.0176x; 1.0171x over previous
"""Trainium2 Bass kernel for nn_PeriodicSetTransformerEncoder.

Design (1.75x over the prior baseline: ~305us vs 538us on 8 cores):
- All projections computed DIRECTLY from x with host-fused weights.
  Since E=128 < D=512, q = x @ ((Wq@wq_W)*sc @ emb_W).T + bq2 etc.
  collapses the matmul contraction dim from 512 to 128: each of
  xe/q/k/v is 4 matmuls (K=128, F=512) per 4-example unit.
- All matmul operands bf16 (FWL weight loads; fp32 PSUM accumulate).
  Measured rel err 6.5e-3 vs the 2e-2 budget.
- Bias folds: k-bias cancels in softmax (adds a per-row constant);
  q-bias host-fused into bq2 (applied on the q PSUM drain); v-bias
  applied as the per-partition ACT bias of the softplus-exp (attention
  rows sum to 1 after renormalization); emb_b on the xe drain; LN
  affine folded into out_W (Wg/c1); cb added on HOST after gather.
- Split PSUM pools so the tile scheduler can overlap units (a single
  shared pool serializes stage allocation in emission order):
  ppj: projections (2-bank tiles, bufs=2) = 4 banks
  pat: scores (1-bank per-w tiles, bufs=2) = 2 banks
  ptl: transpose/att/LN/out (1-bank, bufs=2) = 2 banks
- Engine assignment tuned against the NTFF profile; the wall is the
  per-unit dependency chain threading DVE/ACT (both ~82% busy), so
  chain ops must stay on the fast engines (gpsimd TT is 0.42-eff and
  costs ~16x its extra latency in wall time).

Math (per example, N=128 tokens, E=128, D=512, H=4 heads, hd=128):
  xe = x@Memb.T + emb_b;  q = x@Mq.T + bq2;  k = x@Mk.T;  v = x@Mv.T
  s_h = q_h k_h^T;  e = exp(s);  U = sum_h e_h/rowsum(e_h)
  A = (U * w_j) / rowsum(U * w_j);  att = A @ v
  h = xe + softplus(att + bv2);  out = ((Wg.T h) - c1*mu) * rstd  (+ cb)

Sharding: pure data parallel, batch 512 -> 64 examples per core,
16 units of W=4 examples; features on partitions, tokens on free dim.
"""

import numpy as np

import concourse.bass as bass
import concourse.tile as tile
from concourse import bacc, mybir
from concourse.bass_utils import run_bass_kernel_spmd

F32 = mybir.dt.float32
F32R = mybir.dt.float32r
BF16 = mybir.dt.bfloat16
AX = mybir.AxisListType
OP = mybir.AluOpType
AF = mybir.ActivationFunctionType

B = 512
N = 128
E = 128
D = 512
H = 4
NCORES = 8
BC = B // NCORES          # examples per core
W = 4                     # examples per work unit (free-dim batching)
NU = BC // W              # work units per core
FD = W * N                # moving free dim for the big matmuls (512)


def build_nc(nu=NU):
    nc = bacc.Bacc("TRN2", target_bir_lowering=False, debug=False)

    xg = nc.dram_tensor("xg", [nu, 128, W, N], BF16, kind="ExternalInput").ap()
    wg = nc.dram_tensor("wg", [nu, W, N], BF16, kind="ExternalInput").ap()
    MembT = nc.dram_tensor("MembT", [128, 4, 128], BF16, kind="ExternalInput").ap()
    MqT = nc.dram_tensor("MqT", [128, 4, 128], BF16, kind="ExternalInput").ap()
    MkT = nc.dram_tensor("MkT", [128, 4, 128], BF16, kind="ExternalInput").ap()
    MvT = nc.dram_tensor("MvT", [128, 512], BF16, kind="ExternalInput").ap()
    WgT = nc.dram_tensor("WgT", [128, 4, 128], BF16, kind="ExternalInput").ap()
    c1n = nc.dram_tensor("c1n", [1, 128], BF16, kind="ExternalInput").ap()
    embb = nc.dram_tensor("embb", [128, 4], F32, kind="ExternalInput").ap()
    bq2 = nc.dram_tensor("bq2", [128, 4], F32, kind="ExternalInput").ap()
    bv2 = nc.dram_tensor("bv2", [128, 4], F32, kind="ExternalInput").ap()
    idm = nc.dram_tensor("idm", [128, 128], BF16, kind="ExternalInput").ap()
    onesm = nc.dram_tensor("onesm", [128, 128], BF16, kind="ExternalInput").ap()
    yT = nc.dram_tensor("yT", [nu, 128, W, N], F32, kind="ExternalOutput").ap()

    with tile.TileContext(nc) as tc:
        kernel_body(tc, nu, xg, wg, MembT, MqT, MkT, MvT, WgT, c1n,
                    embb, bq2, bv2, idm, onesm, yT)
    # All transcendentals (exp/ln/square) live in the
    # "natural_log_exp_and_others" activation table set.  Restrict the
    # table map during compile so the act-table-load pass emits a single
    # load instead of thrashing between per-function sets.
    from concourse import hw_specs
    orig = hw_specs.get_activation_tables

    def patched(arch):
        t = orig(arch)
        strip = {AF.Exp, AF.Ln, AF.Square}
        for name, fs in t.items():
            if name != "natural_log_exp_and_others":
                t[name] = fs - strip
        return t

    hw_specs.get_activation_tables = patched
    bacc_mod = __import__("concourse.bacc", fromlist=["get_activation_tables"])
    had = getattr(bacc_mod, "get_activation_tables", None)
    if had is not None:
        bacc_mod.get_activation_tables = patched
    try:
        nc.compile()
    finally:
        hw_specs.get_activation_tables = orig
        if had is not None:
            bacc_mod.get_activation_tables = had
    return nc


def kernel_body(tc, nu, xg, wg, MembT, MqT, MkT, MvT, WgT, c1n,
                embb, bq2, bv2, idm, onesm, yT):
    nc = tc.nc
    from contextlib import ExitStack
    ctx = ExitStack()
    with ctx:
        const = ctx.enter_context(tc.tile_pool(name="const", bufs=1))
        ppj = ctx.enter_context(tc.tile_pool(name="ppj", bufs=2, space="PSUM"))
        pat = ctx.enter_context(tc.tile_pool(name="pat", bufs=2, space="PSUM"))
        ptl = ctx.enter_context(tc.tile_pool(name="ptl", bufs=2, space="PSUM"))
        xpool = ctx.enter_context(tc.tile_pool(name="xpool", bufs=6))
        xepool = ctx.enter_context(tc.tile_pool(name="xepool", bufs=3))
        qkpool = ctx.enter_context(tc.tile_pool(name="qkpool", bufs=4))
        vpool = ctx.enter_context(tc.tile_pool(name="vpool", bufs=4))
        epool = ctx.enter_context(tc.tile_pool(name="epool", bufs=4))
        spool = ctx.enter_context(tc.tile_pool(name="spool", bufs=4))
        hpool = ctx.enter_context(tc.tile_pool(name="hpool", bufs=3))
        tiny = ctx.enter_context(tc.tile_pool(name="tiny", bufs=6))
        opool = ctx.enter_context(tc.tile_pool(name="opool", bufs=4))

        # ---- constants ----
        ident = const.tile([128, 128], BF16)
        nc.sync.dma_start(ident, idm)
        ones = const.tile([128, 128], BF16)
        nc.sync.dma_start(ones, onesm)
        MembT_s = const.tile([128, 4, 128], BF16)
        nc.sync.dma_start(MembT_s, MembT)
        MqT_s = const.tile([128, 4, 128], BF16)
        nc.sync.dma_start(MqT_s, MqT)
        MkT_s = const.tile([128, 4, 128], BF16)
        nc.sync.dma_start(MkT_s, MkT)
        MvT_s = const.tile([128, 512], BF16)
        nc.sync.dma_start(MvT_s, MvT)
        WgT_s = const.tile([128, 4, 128], BF16)
        nc.sync.dma_start(WgT_s, WgT)
        c1n_s = const.tile([1, 128], BF16)
        nc.sync.dma_start(c1n_s, c1n)
        embb_s = const.tile([128, 4], F32)
        nc.sync.dma_start(embb_s, embb)
        bq2_s = const.tile([128, 4], F32)
        nc.sync.dma_start(bq2_s, bq2)
        bv2_s = const.tile([128, 4], F32)
        nc.sync.dma_start(bv2_s, bv2)
        eps = const.tile([128, 1], F32)
        nc.vector.memset(eps, 1e-5)
        one_b = const.tile([128, 1], F32)
        nc.vector.memset(one_b, 1.0)

        for u in range(nu):
            unit_body(nc, u, xg, wg, yT,
                      MembT_s, MqT_s, MkT_s, MvT_s, WgT_s, c1n_s,
                      embb_s, bq2_s, bv2_s, eps, one_b, ident, ones,
                      ppj, pat, ptl, xpool, xepool, qkpool, vpool, epool,
                      spool, hpool, tiny, opool)


def unit_body(nc, u, xg, wg, yT,
              MembT_s, MqT_s, MkT_s, MvT_s, WgT_s, c1n_s,
              embb_s, bq2_s, bv2_s, eps, one_b, ident, ones,
              ppj, pat, ptl, xpool, xepool, qkpool, vpool, epool,
              spool, hpool, tiny, opool):
    ts = bass.ts

    # ---- load x (transposed, bf16) and per-token weights ----
    xT = xpool.tile([128, W, N], BF16, tag="xT")
    nc.sync.dma_start(xT, xg[u])
    wrow = xpool.tile([128, W, N], BF16, tag="wrow")
    nc.gpsimd.dma_start(wrow, wg[u : u + 1].to_broadcast((128, W, N)))

    # ---- projections from x: all K=128 ----
    xeT = xepool.tile([128, 4, FD], BF16, tag="xeT")
    qT = qkpool.tile([128, 4, W, N], BF16, tag="qT")
    kT = qkpool.tile([128, 4, W, N], BF16, tag="kT")
    v = vpool.tile([128, W, 512], BF16, tag="v")
    for hh in range(2):
        pxe = ppj.tile([128, 2, FD], F32, tag="bank", name=f"pxe_{u}_{hh}")
        for c2 in range(2):
            nc.tensor.matmul(pxe[:, c2], MembT_s[:, hh * 2 + c2], xT,
                             start=True, stop=True)
        nc.vector.tensor_add(
            xeT[:, hh * 2 : hh * 2 + 2], pxe,
            embb_s[:, hh * 2 : hh * 2 + 2, None].to_broadcast((128, 2, FD)))
        pq = ppj.tile([128, 2, FD], F32, tag="bank", name=f"pq_{u}_{hh}")
        for c2 in range(2):
            nc.tensor.matmul(pq[:, c2], MqT_s[:, hh * 2 + c2], xT,
                             start=True, stop=True)
        nc.vector.tensor_add(
            qT[:, hh * 2 : hh * 2 + 2], pq,
            bq2_s[:, hh * 2 : hh * 2 + 2, None].to_broadcast((128, 2, FD)))
        pk = ppj.tile([128, 2, FD], F32, tag="bank", name=f"pk_{u}_{hh}")
        for c2 in range(2):
            nc.tensor.matmul(pk[:, c2], MkT_s[:, hh * 2 + c2], xT,
                             start=True, stop=True)
        nc.scalar.copy(kT[:, hh * 2 : hh * 2 + 2], pk)
        pv = ppj.tile([128, 2, 512], F32, tag="bank", name=f"pv_{u}_{hh}")
        for w_i in range(2):
            nc.tensor.matmul(pv[:, w_i], xT[:, hh * 2 + w_i], MvT_s,
                             start=True, stop=True)
        nc.scalar.copy(v[:, hh * 2 : hh * 2 + 2], pv)

    # ---- attention scores + exp ----
    e_all = epool.tile([128, W, H, N], BF16, tag="e_all")
    for w_i in range(W):
        pss = pat.tile([128, H, N], F32, tag="bank", name=f"pss_{u}_{w_i}")
        for h in range(H):
            nc.tensor.matmul(pss[:, h], qT[:, h, w_i],
                             kT[:, h, w_i], start=True, stop=True)
        nc.scalar.activation(e_all[:, w_i], pss, AF.Exp)

    # ---- softmax normalize (per head), head-sum, token weights ----
    s_all = tiny.tile([128, W, H], F32, tag="s_all")
    nc.vector.reduce_sum(s_all, e_all, axis=AX.X)
    r_all = tiny.tile([128, W, H], BF16, tag="r_all")
    with nc.allow_low_precision(reason="softmax normalizer in bf16 is fine"):
        nc.vector.reciprocal(r_all, s_all)
    nc.vector.tensor_mul(e_all, e_all,
                         r_all[:, :, :, None].to_broadcast((128, W, H, N)))
    nc.vector.tensor_add(e_all[:, :, 0:2], e_all[:, :, 0:2], e_all[:, :, 2:4])
    Sw = spool.tile([128, W, N], BF16, tag="Sw")
    nc.vector.tensor_add(Sw, e_all[:, :, 0], e_all[:, :, 1])
    Sww = spool.tile([128, W, N], BF16, tag="Sww")
    nc.vector.tensor_mul(Sww, Sw, wrow)
    dd = tiny.tile([128, W], F32, tag="dd")
    nc.vector.reduce_sum(dd, Sww, axis=AX.X)
    rd = tiny.tile([128, W], BF16, tag="rd")
    with nc.allow_low_precision(reason="attention renormalizer in bf16 is fine"):
        nc.vector.reciprocal(rd, dd)
    Ab = spool.tile([128, W, N], BF16, tag="Ab")
    nc.vector.tensor_mul(Ab, Sww, rd[:, :, None].to_broadcast((128, W, N)))

    # ---- transpose A, att = A @ v  (att^T in PSUM) ----
    pT = ptl.tile([128, W, N], BF16, tag="bank", name=f"pT_{u}")
    for w_i in range(W):
        nc.tensor.transpose(pT[:, w_i], Ab[:, w_i], ident)
    awT = spool.tile([128, W, N], BF16, tag="awT")
    nc.scalar.copy(awT, pT)

    spx = hpool.tile([128, 4, FD], BF16, tag="spx")
    for c in range(4):
        pa = ptl.tile([128, W, N], F32, tag="bank", name=f"pa_{u}_{c}")
        for w_i in range(W):
            nc.tensor.matmul(pa[:, w_i],
                             v[:, w_i, ts(c, 128)], awT[:, w_i],
                             start=True, stop=True)
        # softplus part 1: exp(att + bv2)  (v bias folded in here)
        nc.scalar.activation(spx[:, c], pa, AF.Exp,
                             bias=bv2_s[:, c : c + 1])
    # softplus part 2: ln(exp + 1); then h = xe + softplus
    ea = hpool.tile([128, 4, FD], BF16, tag="ea")
    nc.scalar.activation(ea, spx, AF.Ln, bias=one_b)
    hT = hpool.tile([128, 4, FD], BF16, tag="hT")
    nc.vector.tensor_add(hT, ea, xeT)
    sq = hpool.tile([128, 4, FD], BF16, tag="sq")
    nc.scalar.square(sq, hT)

    # ---- LayerNorm stats over d via ones-matmul ----
    ps_s = ptl.tile([128, FD], F32, tag="bank", name=f"ps_s_{u}")
    for c in range(4):
        nc.tensor.matmul(ps_s, ones, hT[:, c], start=(c == 0), stop=(c == 3))
    ps_q = ptl.tile([128, FD], F32, tag="bank", name=f"ps_q_{u}")
    for c in range(4):
        nc.tensor.matmul(ps_q, ones, sq[:, c], start=(c == 0), stop=(c == 3))
    # mu2 = (s/512)^2 ; var = q/512 - mu2 ; rstd = exp(-0.5*ln(var+eps))
    mu2 = spool.tile([128, FD], F32, tag="mu2")
    nc.scalar.activation(mu2, ps_s, AF.Square, scale=1.0 / D)
    var = spool.tile([128, FD], F32, tag="var")
    nc.vector.scalar_tensor_tensor(out=var, in0=ps_q, scalar=1.0 / D,
                                   in1=mu2, op0=OP.mult, op1=OP.subtract)
    lv = spool.tile([128, FD], F32, tag="lv")
    nc.scalar.activation(lv, var, AF.Ln, bias=eps)
    rstd = spool.tile([128, FD], F32, tag="rstd")
    nc.scalar.activation(rstd, lv, AF.Exp, scale=-0.5)
    m2 = tiny.tile([1, FD], BF16, tag="m2")
    nc.vector.tensor_scalar_mul(m2, ps_s[0:1], 1.0 / D)

    # ---- out^T = Wg-chunks @ h^T - c1 x mu ; then *rstd (cb on host) ----
    po = ptl.tile([128, FD], F32, tag="bank", name=f"po_{u}")
    for c in range(4):
        nc.tensor.matmul(po, WgT_s[:, c], hT[:, c], start=(c == 0), stop=False)
    nc.tensor.matmul(po, c1n_s, m2, start=False, stop=True)
    outT = opool.tile([128, W, N], F32, tag="outT")
    nc.vector.tensor_mul(outT, po, rstd)
    nc.sync.dma_start(yT[u], outT)


# ------------------------- host side -------------------------

def host_prep(x, weights, emb_W, emb_b, wq_W, wq_b, wk_W, wk_b, wv_W, wv_b,
              in_proj_W, in_proj_b, ln_g, ln_b, out_W, out_b):
    """Fuse/reshape parameters and build per-core input maps."""
    import ml_dtypes
    f = np.float32
    bf = ml_dtypes.bfloat16
    sc = 1.0 / np.sqrt(np.float32(E))

    Wq = in_proj_W[:D]
    Wk = in_proj_W[D : 2 * D]
    bqi = in_proj_b[:D]
    Wqc = (Wq @ wq_W) * sc
    bq_eff = (Wq @ wq_b + bqi) * sc
    Wkc = Wk @ wk_W

    Mq = Wqc @ emb_W                       # [D, E]
    bq2v = Wqc @ emb_b + bq_eff            # [D]
    Mk = Wkc @ emb_W
    Mv = wv_W @ emb_W
    bv2v = wv_W @ emb_b + wv_b             # [D]

    Wg = out_W.T * ln_g[:, None]           # [D, E]
    c1 = Wg.sum(axis=0)                    # [E]
    cbv = out_b + out_W @ ln_b             # [E]

    def chunkT(M):
        # M [D, E] applied as x @ M.T -> stationary chunks [128(E), 4, 128]
        return np.ascontiguousarray(M.T.reshape(128, 4, 128)).astype(bf)

    def pcol(vec):
        return np.ascontiguousarray(vec.reshape(4, 128).T).astype(f)

    params = {
        "MembT": np.ascontiguousarray(emb_W.T.reshape(128, 4, 128)).astype(bf),
        "MqT": chunkT(Mq),
        "MkT": chunkT(Mk),
        "MvT": np.ascontiguousarray(Mv.T).astype(bf),   # [E, D] moving
        "WgT": np.ascontiguousarray(
            Wg.reshape(4, 128, 128).transpose(1, 0, 2)).astype(bf),
        "c1n": np.ascontiguousarray((-c1).reshape(1, 128)).astype(bf),
        "embb": pcol(emb_b),
        "bq2": pcol(bq2v),
        "bv2": pcol(bv2v),
        "onesm": np.ones((128, 128), dtype=bf),
        "idm": np.eye(128).astype(bf),
    }

    in_maps = []
    for c in range(NCORES):
        xs = x[c * BC : (c + 1) * BC].astype(f)
        ws = weights[c * BC : (c + 1) * BC, :, 0].astype(f)
        xgc = np.ascontiguousarray(
            xs.reshape(NU, W, N, E).transpose(0, 3, 1, 2)).astype(bf)
        wgc = np.ascontiguousarray(ws.reshape(NU, W, N)).astype(bf)
        m = dict(params)
        m["xg"] = xgc
        m["wg"] = wgc
        in_maps.append(m)
    return in_maps, cbv


_NC_CACHE = {}


def kernel(**inputs):
    key = "full"
    if key not in _NC_CACHE:
        _NC_CACHE[key] = build_nc(NU)
    nc = _NC_CACHE[key]
    in_maps, cbv = host_prep(**inputs)
    res = run_bass_kernel_spmd(nc, in_maps, core_ids=list(range(NCORES)))
    outs = []
    for c in range(NCORES):
        yt = res.results[c]["yT"]                  # [NU, 128(E), W, N]
        y = yt.transpose(0, 2, 3, 1).reshape(BC, N, E)
        outs.append(y)
    full = np.concatenate(outs, axis=0) + cbv[None, None, :]
    return np.ascontiguousarray(full).astype(np.float32)
